# revision 18
# baseline (speedup 1.0000x reference)
"""Context-segment scoring kernel for Trainium2 (Bass/Tile).

Computes out[b, n] = sum_e c[b, n, e] * s[b, e] for
c = c_embeds [32, 32, 32, 8, 256] viewed as [B=32, N=8192, E=256] and
s = s_embeds [32, 256].

Strategy: cast inputs to fp16 on the host (quantization rel-err ~3e-4,
far under the 2e-2 gate) and transpose c to [B, E, N] so the
TensorEngine does the entire multiply-reduce as matvecs:
  psum[1, 512] += s_chunk[128, 1].T @ cT_chunk[128, 512]
accumulated over the two 128-wide E chunks. fp16 halves HBM traffic
(16 MiB/core); the PE replaces all the DVE/ScalarE elementwise work
that bounded the v1 kernel at ~121 us.

Schedule (measured-driven):
- All input DMAs ride the single SP HWDGE ring as 1 MiB chunk slices,
  issued in exact consumption order (c0 then c1 per block). The FIFO
  ring drains across all 16 SDMA engines at ~420 GB/s (97% of the
  435 GB/s SBUF-AXI fabric ceiling), and arrival order == consumption
  order, so the PE never waits on an out-of-order slice. (Splitting
  across two HWDGE rings or SWDGE, or merging chunks into 2 MiB DMAs,
  all measured slower: out-of-order arrivals, ~4.3 us Q7 drains, or a
  lumpier stream.)
- The final block is graded (3072/1024 n) so the work that depends on
  the last-arriving bytes is small; grading finer costs more DMAs,
  which trips the ~8-deep DMA issue window and delays the tail slices
  past the stream end (measured on 22-DMA variants).
- Weights load once per chunk-group; matmuls stream back-to-back and
  keep the PE at the warm 2.4 GHz HAM state.
- PSUM is drained in [1, 1024] copies alternating DVE/ScalarE
  (different banks - legal in parallel); outputs ride the otherwise
  idle ACT HWDGE ring (on the SP ring they would queue behind
  megabytes of input and stall extraction via the out-tile WAR).

Sharding: data-parallel over batch - 8 NeuronCores, 4 batches each.
"""

import numpy as np

import concourse.bacc as bacc
import concourse.bass as bass
import concourse.mybir as mybir
import concourse.tile as tile
from concourse.bass_utils import run_bass_kernel_spmd

B, N, E = 32, 8192, 256
NCORES = 8
B_LOC = B // NCORES          # 4 batches per core
P = 128                      # SBUF partitions / PE contract dim
ECH = E // P                 # 2 e-chunks of 128
NT = 512                     # n per matmul (one PSUM bank of fp32)
NSLICE = 4096                # n per full input segment (1 MiB fp16 per chunk)
NSL = N // NSLICE            # full segments per batch
PSG = 2                      # n-tiles per psum tile ([1, 1024] = 2 banks)
TAIL = (3072, 1024)          # grading of the final segment

F32 = mybir.dt.float32
F16 = mybir.dt.float16


def build_body(tc, out_ap, c_ap, s_ap):
    """Per-core Tile program. DRAM access patterns:
    out [B_LOC, N] f32, c [B_LOC, ECH, P, N] f16, s [P, B_LOC*ECH] f16."""
    nc = tc.nc
    with (
        tc.tile_pool(name="sseg", bufs=1) as s_pool,
        tc.tile_pool(name="cin", bufs=2 * (B_LOC - 1)) as cin_pool,
        tc.tile_pool(name="cin_s", bufs=6) as cins_pool,
        tc.tile_pool(name="oacc", bufs=3) as out_pool,
        tc.tile_pool(name="ps", bufs=4, space="PSUM") as ps_pool,
    ):
        # All segment-embedding columns in one DMA: s_all[:, b*ECH+k] is the
        # [128, 1] stationary operand for (batch b, e-chunk k).
        s_all = s_pool.tile([P, B_LOC * ECH], F16, tag="s", name="s_all")
        nc.sync.dma_start(s_all[:, :], s_ap)

        # DMA plan, decoupled from compute waves: batches 0..2 load as one
        # 2 MiB DMA per chunk (minimal issue count - stays far under the
        # ~8-deep DMA issue window, smoothest possible stream), batch 3
        # loads in fine slices with a graded tail so the work depending
        # on the last-arriving bytes is tiny. Issue order == consumption
        # order on the FIFO SP ring.
        # Each compute wave: (b, n0_global, ln, c0_tile, c1_tile, off).
        waves = []
        for b in range(B_LOC - 1):
            t0 = cin_pool.tile([P, N], F16, tag="cin", name="t0")
            nc.sync.dma_start(t0[:], c_ap[b, 0, :, :])
            t1 = cin_pool.tile([P, N], F16, tag="cin", name="t1")
            nc.sync.dma_start(t1[:], c_ap[b, 1, :, :])
            for h in range(NSL):
                waves.append((b, h * NSLICE, NSLICE, t0, t1, h * NSLICE))
        b = B_LOC - 1
        for n0, ln in ((0, NSLICE), (NSLICE, TAIL[0]), (NSLICE + TAIL[0], TAIL[1])):
            s0 = cins_pool.tile([P, NSLICE], F16, tag="cin_s", name="s0")
            nc.sync.dma_start(s0[:, :ln], c_ap[b, 0, :, n0:n0 + ln])
            s1 = cins_pool.tile([P, NSLICE], F16, tag="cin_s", name="s1")
            nc.sync.dma_start(s1[:, :ln], c_ap[b, 1, :, n0:n0 + ln])
            waves.append((b, n0, ln, s0, s1, 0))

        eng = 0
        for b, n0, ln, c0, c1, off in waves:
            ntiles = ln // NT
            npt = (ntiles + PSG - 1) // PSG
            pts = [
                ps_pool.tile([1, PSG * NT], F32, tag="pt", name=f"pt{g}")
                for g in range(npt)
            ]
            # All chunk-0 matmuls share one weight load, then all chunk-1.
            for k, ct in ((0, c0), (1, c1)):
                w = s_all[:, b * ECH + k: b * ECH + k + 1]
                start, stop = (k == 0), (k == ECH - 1)
                for t in range(ntiles):
                    nc.tensor.matmul(
                        pts[t // PSG][0:1, (t % PSG) * NT:(t % PSG + 1) * NT],
                        w,
                        ct[:, off + t * NT: off + (t + 1) * NT],
                        start=start,
                        stop=stop,
                    )

            ot = out_pool.tile([1, NSLICE], F32, tag="ot", name="ot")
            for g in range(npt):
                gl = min(PSG * NT, ln - g * PSG * NT)
                dst = ot[0:1, g * PSG * NT: g * PSG * NT + gl]
                if eng % 2 == 0:
                    nc.vector.tensor_copy(dst, pts[g][0:1, :gl])
                else:
                    nc.scalar.copy(dst, pts[g][0:1, :gl])
                eng += 1
            nc.scalar.dma_start(
                out_ap[b, n0:n0 + ln].unsqueeze(0), ot[0:1, :ln]
            )


_NC_CACHE = None


def _get_nc():
    global _NC_CACHE
    if _NC_CACHE is None:
        nc = bacc.Bacc(
            "TRN2",
            target_bir_lowering=False,
            debug=False,
            num_devices=NCORES,
        )
        c = nc.dram_tensor("c", [B_LOC, ECH, P, N], F16, kind="ExternalInput")
        s = nc.dram_tensor("s", [P, B_LOC * ECH], F16, kind="ExternalInput")
        o = nc.dram_tensor("o", [B_LOC, N], F32, kind="ExternalOutput")
        with tile.TileContext(nc) as tc:
            build_body(tc, o.ap(), c.ap(), s.ap())
        nc.compile()
        _NC_CACHE = nc
    return _NC_CACHE


def _run(c_embeds: np.ndarray, s_embeds: np.ndarray, **kwargs):
    c = np.asarray(c_embeds, dtype=np.float32).reshape(B, N, E)
    # [B, N, E] -> [B, E, N] fp16, chunked: [B, ECH, P, N]
    ct = np.ascontiguousarray(
        c.astype(np.float16).transpose(0, 2, 1)
    ).reshape(B, ECH, P, N)
    # s[b, e] -> per-core [P, B_LOC*ECH] with column (b*ECH+k) = s[b, 128k:128k+128]
    s = np.asarray(s_embeds, dtype=np.float32).astype(np.float16)
    s = s.reshape(B, ECH, P)
    nc = _get_nc()
    in_maps = [
        {
            "c": ct[k * B_LOC:(k + 1) * B_LOC],
            "s": np.ascontiguousarray(
                s[k * B_LOC:(k + 1) * B_LOC].reshape(B_LOC * ECH, P).T
            ),
        }
        for k in range(NCORES)
    ]
    r = run_bass_kernel_spmd(nc, in_maps, core_ids=list(range(NCORES)), **kwargs)
    out = np.concatenate([r.results[k]["o"] for k in range(NCORES)], axis=0)
    return out.astype(np.float32), r


def kernel(c_embeds: np.ndarray, s_embeds: np.ndarray) -> np.ndarray:
    out, _ = _run(c_embeds, s_embeds)
    return out


# revision 19
# speedup vs baseline: 1.0599x; 1.0599x over previous
"""Context-segment scoring kernel for Trainium2 (Bass/Tile).

Computes out[b, n] = sum_e c[b, n, e] * s[b, e] for
c = c_embeds [32, 32, 32, 8, 256] viewed as [B=32, N=8192, E=256] and
s = s_embeds [32, 256].

Strategy: cast inputs to fp16 on the host (quantization rel-err ~3e-4,
far under the 2e-2 gate) and transpose c to [B, E, N] so the
TensorEngine does the entire multiply-reduce as matvecs:
  psum[1, 512] += s_chunk[128, 1].T @ cT_chunk[128, 512]
accumulated over the two 128-wide E chunks. fp16 halves HBM traffic
(16 MiB/core); the PE replaces all the DVE/ScalarE elementwise work
that bounded the v1 kernel at ~121 us.

Schedule (measured-driven; several variants benched, this one best):
- All input DMAs ride the single SP HWDGE ring as 1 MiB chunk slices,
  issued back-to-back in exact consumption order (c0 then c1 per
  block). The FIFO ring drains across all 16 SDMA engines at
  ~420 GB/s (97% of the 435 GB/s SBUF-AXI fabric ceiling), and
  arrival order == consumption order, so the PE never waits on an
  out-of-order slice. Variants that split across two HWDGE rings or
  SWDGE (out-of-order arrivals, ~4.3 us Q7 drains), merged both
  chunks per DMA (lumpier stream, later PE start), or graded the tail
  with extra small DMAs (trips the ~8-deep DMA issue window) all
  measured 5-12 us slower.
- One buffer per slice - no WAR wait ever blocks a DMA issue.
- Weights load once per 8-matmul chunk-group; matmuls stream
  back-to-back and keep the PE at the warm 2.4 GHz HAM state.
- PSUM is drained in [1, 1024] copies alternating DVE/ScalarE
  (different banks - legal in parallel); outputs ride the otherwise
  idle ACT HWDGE ring (on the SP ring they would queue behind
  megabytes of input and stall extraction via the out-tile WAR).

Sharding: data-parallel over batch - 8 NeuronCores, 4 batches each.
"""

import numpy as np

import concourse.bacc as bacc
import concourse.bass as bass
import concourse.mybir as mybir
import concourse.tile as tile
from concourse.bass_utils import run_bass_kernel_spmd

B, N, E = 32, 8192, 256
NCORES = 8
B_LOC = B // NCORES          # 4 batches per core
P = 128                      # SBUF partitions / PE contract dim
ECH = E // P                 # 2 e-chunks of 128
NT = 512                     # n per matmul (one PSUM bank of fp32)
NSLICE = 4096                # n per input DMA slice (1 MiB fp16)
NSL = N // NSLICE            # slices per (batch, chunk)
TPB = NSLICE // NT           # 8 matmul n-tiles per block
PSG = 2                      # n-tiles per psum tile ([1, 1024] = 2 banks)

F32 = mybir.dt.float32
F16 = mybir.dt.float16


def build_body(tc, out_ap, c_ap, s_ap):
    """Per-core Tile program. DRAM access patterns:
    out [B_LOC, N] f32, c [B_LOC, ECH, P, N] f16, s [P, B_LOC*ECH] f16."""
    nc = tc.nc
    with (
        tc.tile_pool(name="sseg", bufs=1) as s_pool,
        tc.tile_pool(name="cin", bufs=2 * NSL * B_LOC) as cin_pool,
        tc.tile_pool(name="oacc", bufs=3) as out_pool,
        tc.tile_pool(name="ps", bufs=4, space="PSUM") as ps_pool,
    ):
        # All segment-embedding columns in one DMA: s_all[:, b*ECH+k] is the
        # [128, 1] stationary operand for (batch b, e-chunk k).
        s_all = s_pool.tile([P, B_LOC * ECH], F16, tag="s", name="s_all")
        nc.sync.dma_start(s_all[:, :], s_ap)

        # Pre-issue every input DMA on the single SP HWDGE ring, in exact
        # consumption order (c0 then c1 per block).
        ctiles = {}
        for b in range(B_LOC):
            for h in range(NSL):
                c0 = cin_pool.tile([P, NSLICE], F16, tag="cin", name="c0")
                nc.sync.dma_start(c0[:], c_ap[b, 0, :, h * NSLICE:(h + 1) * NSLICE])
                c1 = cin_pool.tile([P, NSLICE], F16, tag="cin", name="c1")
                nc.sync.dma_start(c1[:], c_ap[b, 1, :, h * NSLICE:(h + 1) * NSLICE])
                ctiles[b, h] = (c0, c1)

        for b in range(B_LOC):
            for h in range(NSL):
                c0, c1 = ctiles[b, h]
                pts = [
                    ps_pool.tile([1, PSG * NT], F32, tag="pt", name=f"pt{g}")
                    for g in range(TPB // PSG)
                ]
                # All chunk-0 matmuls share one weight load, then all chunk-1.
                for k, ct, start, stop in ((0, c0, True, False), (1, c1, False, True)):
                    w = s_all[:, b * ECH + k: b * ECH + k + 1]
                    for t in range(TPB):
                        nc.tensor.matmul(
                            pts[t // PSG][0:1, (t % PSG) * NT:(t % PSG + 1) * NT],
                            w,
                            ct[:, t * NT:(t + 1) * NT],
                            start=start,
                            stop=stop,
                        )

                ot = out_pool.tile([1, NSLICE], F32, tag="ot", name="ot")
                for g in range(TPB // PSG):
                    dst = ot[0:1, g * PSG * NT:(g + 1) * PSG * NT]
                    if g % 2 == 0:
                        nc.vector.tensor_copy(dst, pts[g][:, :])
                    else:
                        nc.scalar.copy(dst, pts[g][:, :])
                # Output rides the ACT HWDGE ring: the SP ring is FIFO and
                # still holds megabytes of queued input - an out-DMA there
                # would not drain (and via the out-tile WAR would stall
                # extraction, PSUM reuse, and ultimately the PE).
                nc.scalar.dma_start(
                    out_ap[b, h * NSLICE:(h + 1) * NSLICE].unsqueeze(0), ot[:, :]
                )


_NC_CACHE = None


def _get_nc():
    global _NC_CACHE
    if _NC_CACHE is None:
        nc = bacc.Bacc(
            "TRN2",
            target_bir_lowering=False,
            debug=False,
            num_devices=NCORES,
        )
        c = nc.dram_tensor("c", [B_LOC, ECH, P, N], F16, kind="ExternalInput")
        s = nc.dram_tensor("s", [P, B_LOC * ECH], F16, kind="ExternalInput")
        o = nc.dram_tensor("o", [B_LOC, N], F32, kind="ExternalOutput")
        with tile.TileContext(nc) as tc:
            build_body(tc, o.ap(), c.ap(), s.ap())
        nc.compile()
        _NC_CACHE = nc
    return _NC_CACHE


def _run(c_embeds: np.ndarray, s_embeds: np.ndarray, **kwargs):
    c = np.asarray(c_embeds, dtype=np.float32).reshape(B, N, E)
    # [B, N, E] -> [B, E, N] fp16, chunked: [B, ECH, P, N]
    ct = np.ascontiguousarray(
        c.astype(np.float16).transpose(0, 2, 1)
    ).reshape(B, ECH, P, N)
    # s[b, e] -> per-core [P, B_LOC*ECH] with column (b*ECH+k) = s[b, 128k:128k+128]
    s = np.asarray(s_embeds, dtype=np.float32).astype(np.float16)
    s = s.reshape(B, ECH, P)
    nc = _get_nc()
    in_maps = [
        {
            "c": ct[k * B_LOC:(k + 1) * B_LOC],
            "s": np.ascontiguousarray(
                s[k * B_LOC:(k + 1) * B_LOC].reshape(B_LOC * ECH, P).T
            ),
        }
        for k in range(NCORES)
    ]
    r = run_bass_kernel_spmd(nc, in_maps, core_ids=list(range(NCORES)), **kwargs)
    out = np.concatenate([r.results[k]["o"] for k in range(NCORES)], axis=0)
    return out.astype(np.float32), r


def kernel(c_embeds: np.ndarray, s_embeds: np.ndarray) -> np.ndarray:
    out, _ = _run(c_embeds, s_embeds)
    return out
